# revision 1
# baseline (speedup 1.0000x reference)
"""Trainium2 Bass kernel for the DFS-Mixer style-attention module.

Computation (per batch b):
    dot[k,c]  = sum_hw CT[c,hw] * CR[k,c,hw]
    norm[k,c] = sqrt(sum_hw CR[k,c,hw]^2)
    w[.,c]    = softmax_k(2 * dot[.,c] / norm[.,c])
    out[c,hw] = sum_k IR[k,c,hw] * w[k,c]

Sharding: data-parallel over batch B=8 across the 8 NeuronCores (one b per
core, no cross-core communication).

Per-core layout: C=256 tiled as 2 x 128 SBUF partitions, HW=4096 on the free
axis.  The kernel streams 71.3 MB in / 4.2 MB out per core at the ~390 GB/s
per-core DMA ceiling (16 engines x 24.6 GB/s at 16 KB packets), so the
design keeps the engines fed at all times:

- Full [128, 4096] tile DMAs everywhere (16 KB per-row packets are ~7%
  cheaper per byte than smaller ones), except the very last k of the last
  c-tile which arrives as quarters to shrink the end-of-kernel tail.
- The two c-tiles' work is INTERLEAVED on the in-order sync queue:
  [CT, CR(t0,*), then alternating CR(t1,k) / IR(t0,k), then IR(t1,*)].
  Phase 1 consumers (ACT+DVE) and phase 3's TensorE then drain two
  independent streams concurrently, so a hiccup in either consumer never
  lets the DMA queues run dry, and TensorE starts ~70 us earlier than a
  strictly phased order would allow.
- Phase 1, per CR tile: ACT does ||CR||^2 via one full-tile
  Square-with-accumulate (elementwise output into a dead SBUF sink -- NOT
  PSUM, which would chain phase 3's first matmul behind all of phase 1);
  DVE does the dot via one fused scalar_tensor_tensor, written in-place
  over the CR tile.  This split keeps BOTH engines ~10-25% under the
  5.3 us/tile DMA cadence, which matters because the slot-release ->
  next-issue loop turns any consumer transient into a convoy.
- Phase 3: TensorE handles ALL k as float32r matmuls (1 cycle/row, 4x
  faster than fp32's LOW/HIGH 2-pass) with the 128x128 diagonal
  diag(w[:,k]) accumulating across k in PSUM.  ACT (which has slack and,
  unlike DVE, is not on the CR slot-release path) drains finished PSUM
  quarters to SBUF staging; outputs ride the scalar queue so the sync
  queue never head-of-line blocks on epilogue dependencies.
"""

import os
import sys

import numpy as np


def _import_concourse():
    try:
        import concourse.bass  # noqa: F401
    except ImportError:
        for p in ("/opt/trn_rl_repo", "/root/.axon_site/_ro/trn_rl_repo"):
            if os.path.isdir(p) and p not in sys.path:
                sys.path.insert(0, p)
        import concourse.bass  # noqa: F401


_import_concourse()

import concourse.bass as bass  # noqa: E402
import concourse.mybir as mybir  # noqa: E402
from concourse import tile  # noqa: E402
from concourse.bass_utils import run_bass_kernel_spmd  # noqa: E402
from concourse.vector_clock import ScopedClock, VectorClock  # noqa: E402


def _split_multiwait_bir(bir: bytes) -> bytes:
    """The neuronxcc walrus in this container encodes at most ONE sync-wait
    per instruction; Tile emits several.  Hoist extra waits onto same-engine
    NoOp instructions inserted immediately before the original instruction
    (engines execute in order, so waiting earlier on the same engine is
    semantically identical).  Sem *updates* are left untouched (a DMA's
    completion-inc cannot move to a sequencer NoOp)."""
    import json

    j = json.loads(bir)
    ctr = 0
    for f in j.get("functions", []):
        for bb in f.get("blocks", []):
            out_insts = []
            for ins in bb.get("instructions", []):
                si = ins.get("sync_info")
                waits = (si or {}).get("on_wait") or []
                if len(waits) > 1:
                    for w in waits[:-1]:
                        ctr += 1
                        nop = {
                            "engine": ins["engine"],
                            "ins": [],
                            "outs": [],
                            "name": f"waitsplit-{ctr}",
                            "opcode": "NoOp",
                            "sync_info": {"on_update": [], "on_wait": [w]},
                        }
                        if "debug" in ins:
                            nop["debug"] = ins["debug"]
                        out_insts.append(nop)
                    si["on_wait"] = [waits[-1]]
                out_insts.append(ins)
            bb["instructions"] = out_insts
    return json.dumps(j).encode()


_orig_to_json_bytes = bass.Bass.to_json_bytes


def _patched_to_json_bytes(self, *a, **kw):
    return _split_multiwait_bir(_orig_to_json_bytes(self, *a, **kw))


bass.Bass.to_json_bytes = _patched_to_json_bytes


def _patched_drain_and_barrier(self, tick_clock, wait_clock):
    # Stock TileContext exit emits one Drain waiting on every used semaphore,
    # which this walrus rejects ("Too many sync wait commands").  Emit one
    # Drain per semaphore instead.
    gc = tick_clock.global_clock
    n = len(gc)
    nonzero = [p for p in range(n) if gc[p] > 0] or [0]
    for p in nonzero:
        d = self.nc.sync.drain()
        vec = [gc[q] if q == p else 0 for q in range(n)]
        wait_clock.add_sem_waits(d.ins, ScopedClock({None: VectorClock(vec)}))
    self.nc.all_engine_barrier()
    popped = self.nc._tile_sem_poison_stack.pop()
    assert popped is self._sem_poison
    self.nc.clear_and_free_semaphores(list(self.sems.allocated().values()))
    self.nc.all_engine_barrier()


tile.TileContext._drain_and_barrier = _patched_drain_and_barrier

FP = mybir.dt.float32
F32R = mybir.dt.float32r
B, K, C, H, W = 8, 8, 256, 64, 64
HW = H * W
P = 128                 # SBUF partitions
NCT = C // P            # 2 c-tiles per core
MMN = 512               # moving free dim per matmul (= one PSUM bank of f32)
QN = 1024               # tail quarter width (last k of last c-tile)
HH = 2048               # consumer half-tile width

_AF = mybir.ActivationFunctionType
_OP = mybir.AluOpType
_X = mybir.AxisListType.X


def build_nc() -> bass.Bass:
    nc = bass.Bass()
    # float32r: same bits as fp32, but matmul runs single-pass (1 cyc/row,
    # 4x fp32).  The BIR verifier requires every producer feeding an f32r
    # matmul to carry the f32r dtype, so IR is declared f32r at the DRAM
    # parameter (marshals as np.float32).
    IR = nc.declare_dram_parameter("IR", [K, C, HW], F32R, isOutput=False)
    CR = nc.declare_dram_parameter("CR", [K, C, HW], FP, isOutput=False)
    CT = nc.declare_dram_parameter("CT", [C, HW], FP, isOutput=False)
    OUT = nc.declare_dram_parameter("OUT", [C, HW], FP, isOutput=True)

    with tile.TileContext(nc) as tc:
        with (
            tc.tile_pool(name="ctp", bufs=1) as ct_pool,
            tc.tile_pool(name="crp", bufs=4) as cr_pool,
            tc.tile_pool(name="irp", bufs=4) as ir_pool,
            tc.tile_pool(name="ir7", bufs=2) as ir7_pool,
            tc.tile_pool(name="obp", bufs=3) as ob_pool,
            tc.tile_pool(name="snk", bufs=1) as sink_pool,
            tc.tile_pool(name="sml", bufs=1) as small,
            tc.tile_pool(name="psp", bufs=1, space="PSUM") as psum_pool,
        ):
            acc = psum_pool.tile([P, HW], FP, name="acc")
            # Dead destination for ACT's Square elementwise output.  Only
            # ever written, only by ACT, so it gates nothing.
            sink = sink_pool.tile([P, HH], FP, name="sink")

            # Diagonal ones mask, built once: mask[p, f] = (p == f).
            ones_t = small.tile([P, P], FP, name="ones_t")
            nc.vector.memset(ones_t[:], 1.0)
            mask = small.tile([P, P], FP, name="mask")
            nc.gpsimd.affine_select(
                mask[:],
                ones_t[:],
                pattern=[[-1, P]],
                compare_op=_OP.is_equal,
                fill=0.0,
                base=0,
                channel_multiplier=1,
            )

            # Content-target features stay resident in SBUF (reused by all
            # k).  Issued from the scalar queue so they overlap the sync
            # queue's first CR issues.
            ct_tiles = []
            for t in range(NCT):
                ctt = ct_pool.tile([P, HW], FP, name=f"ct{t}", tag=f"ct{t}")
                nc.scalar.dma_start(out=ctt[:], in_=CT[t * P:(t + 1) * P, :])
                ct_tiles.append(ctt)

            dot2s, sq2s, ws, wms = [], [], [None] * NCT, [None] * NCT

            def phase1_k(t, k):
                """Load CR[k] for c-tile t; sq on ACT + fused dot on DVE,
                per [128, 2048] half (HW runs both engines measurably
                faster on 2048-wide ops than on 4096-wide ones).

                CR rides the sync queue; IR rides the GpSimd SWDGE queue;
                CT/outputs ride the scalar queue.  Three independent
                in-order issue queues mean a full slot pool in one stream
                never head-of-line blocks the others."""
                cs = slice(t * P, (t + 1) * P)
                crt = cr_pool.tile([P, HW], FP, name="crt", tag="cr")
                nc.sync.dma_start(out=crt[:], in_=CR[k, cs, :])
                for h in range(HW // HH):
                    col = k * 2 + h
                    hs = slice(h * HH, (h + 1) * HH)
                    nc.scalar.activation(
                        out=sink[:], in_=crt[:, hs], func=_AF.Square,
                        accum_out=sq2s[t][:, col:col + 1],
                    )
                    nc.vector.scalar_tensor_tensor(
                        out=crt[:, hs], in0=crt[:, hs], scalar=1.0,
                        in1=ct_tiles[t][:, hs],
                        op0=_OP.mult, op1=_OP.mult,
                        accum_out=dot2s[t][:, col:col + 1],
                    )

            def softmax(t):
                """Combine half-accumulators, softmax over K, build diag
                weight matrices (tiny [128, K]/[128, 128] ops)."""
                dot2, sq2 = dot2s[t], sq2s[t]
                dot = small.tile([P, K], FP, name=f"dotc{t}", tag=f"dotc{t}")
                sq = small.tile([P, K], FP, name=f"sqc{t}", tag=f"sqc{t}")
                nc.vector.tensor_add(
                    dot[:], dot2[:, 0:2 * K:2], dot2[:, 1:2 * K:2]
                )
                nc.vector.tensor_add(
                    sq[:], sq2[:, 0:2 * K:2], sq2[:, 1:2 * K:2]
                )
                norm = small.tile([P, K], FP, name=f"norm{t}", tag=f"norm{t}")
                nc.scalar.activation(norm[:], sq[:], func=_AF.Sqrt)
                rnorm = small.tile([P, K], FP, name=f"rnorm{t}", tag=f"rn{t}")
                nc.vector.reciprocal(rnorm[:], norm[:])
                sim = small.tile([P, K], FP, name=f"sim{t}", tag=f"sim{t}")
                nc.vector.tensor_mul(sim[:], dot[:], rnorm[:])
                mx = small.tile([P, 1], FP, name=f"mx{t}", tag=f"mx{t}")
                nc.vector.reduce_max(mx[:], sim[:], axis=_X)
                nbias = small.tile([P, 1], FP, name=f"nb{t}", tag=f"nb{t}")
                nc.vector.tensor_scalar_mul(nbias[:], mx[:], -2.0)
                e = small.tile([P, K], FP, name=f"e{t}", tag=f"e{t}")
                nc.scalar.activation(
                    e[:], sim[:], func=_AF.Exp, bias=nbias[:, 0:1], scale=2.0
                )
                s = small.tile([P, 1], FP, name=f"s{t}", tag=f"s{t}")
                nc.vector.reduce_sum(s[:], e[:], axis=_X)
                rs = small.tile([P, 1], FP, name=f"rs{t}", tag=f"rs{t}")
                nc.vector.reciprocal(rs[:], s[:])
                w = small.tile([P, K], FP, name=f"w{t}", tag=f"w{t}")
                nc.vector.tensor_scalar_mul(w[:], e[:], rs[:, 0:1])
                ws[t] = w
                wmt = []
                for k in range(K):
                    wm = small.tile(
                        [P, P], F32R, name=f"wm{t}{k}", tag=f"wm{t}{k}"
                    )
                    nc.vector.tensor_scalar_mul(wm[:], mask[:], w[:, k:k + 1])
                    wmt.append(wm)
                wms[t] = wmt

            def drain(t, q, on_dve=False):
                """Copy a finished PSUM quarter to staging (ACT, or DVE for
                alternating tail quarters so the final drains run on two
                engines in parallel); the output DMA streams it out from
                the scalar queue."""
                cs = slice(t * P, (t + 1) * P)
                qs = slice(q * QN, (q + 1) * QN)
                ob = ob_pool.tile([P, QN], FP, name="ob", tag="ob")
                if on_dve:
                    nc.vector.tensor_scalar_add(ob[:], acc[:, qs], 0.0)
                else:
                    nc.scalar.activation(ob[:], acc[:, qs], func=_AF.Copy)
                nc.scalar.dma_start(out=OUT[cs, qs], in_=ob[:])

            def phase3_k(t, k):
                """Load IR[k] for c-tile t and fold it into the PSUM
                accumulator with f32r diagonal matmuls.  The last k closes
                each bank group and drains it."""
                cs = slice(t * P, (t + 1) * P)
                last = k == K - 1
                if not (last and t == NCT - 1):
                    irt = ir_pool.tile([P, HW], F32R, name="irt", tag="ir")
                    nc.gpsimd.dma_start(out=irt[:], in_=IR[k, cs, :])
                    for j in range(HW // MMN):
                        col = j * MMN
                        nc.tensor.matmul(
                            acc[:, col:col + MMN],
                            wms[t][k][:],
                            irt[:, col:col + MMN],
                            start=(k == 0),
                            stop=last,
                        )
                    if last:
                        for q in range(HW // QN):
                            drain(t, q)
                else:
                    # Very last input: quarter loads on the sync queue
                    # (empty once CR is done -- the gpsimd queue caps
                    # outstanding transfers, which would strand the tail
                    # quarters behind earlier in-flight tiles) so the tail
                    # after the final byte is two short matmuls + copy +
                    # small DMA, with drains alternating across engines.
                    ir7s = []
                    for q in range(HW // QN):
                        qs = slice(q * QN, (q + 1) * QN)
                        ir7 = ir7_pool.tile([P, QN], F32R, name="ir7", tag="ir7")
                        nc.gpsimd.dma_start(out=ir7[:], in_=IR[K - 1, cs, qs])
                        ir7s.append(ir7)
                    for q in range(HW // QN):
                        for jj in range(QN // MMN):
                            col = q * QN + jj * MMN
                            nc.tensor.matmul(
                                acc[:, col:col + MMN],
                                wms[t][K - 1][:],
                                ir7s[q][:, jj * MMN:(jj + 1) * MMN],
                                start=False,
                                stop=True,
                            )
                        drain(t, q)

            for t in range(NCT):
                dot2s.append(
                    small.tile([P, 2 * K], FP, name=f"dot{t}", tag=f"dot{t}")
                )
                sq2s.append(
                    small.tile([P, 2 * K], FP, name=f"sq{t}", tag=f"sq{t}")
                )

            # ---- Interleaved schedule ----
            # CR(t1) / IR(t0) alternate on the sync queue: phase 1's ACT+DVE
            # and phase 3's TensorE drain two independent streams, so a
            # transient in either consumer never idles the DMA engines.
            for k in range(K):
                phase1_k(0, k)
            phase1_k(1, 0)
            softmax(0)
            for k in range(1, K):
                phase3_k(0, k - 1)
                phase1_k(1, k)
            phase3_k(0, K - 1)
            softmax(1)
            for k in range(K):
                phase3_k(1, k)

    return nc


_NC_CACHE = None


def _get_nc() -> bass.Bass:
    global _NC_CACHE
    if _NC_CACHE is None:
        _NC_CACHE = build_nc()
    return _NC_CACHE


def run(inputs: dict, trace: bool = False):
    """Shard over B, run on 8 cores, gather. Returns (output, BassKernelResults)."""
    ir = np.ascontiguousarray(np.asarray(inputs["IR_features"], dtype=np.float32))
    cr = np.ascontiguousarray(np.asarray(inputs["CR_features"], dtype=np.float32))
    ct = np.ascontiguousarray(np.asarray(inputs["CT_feature"], dtype=np.float32))
    assert ir.shape == (B, K, C, H, W) and cr.shape == (B, K, C, H, W)
    assert ct.shape == (B, C, H, W)

    in_maps = [
        {
            "IR": ir[b].reshape(K, C, HW),
            "CR": cr[b].reshape(K, C, HW),
            "CT": ct[b].reshape(C, HW),
        }
        for b in range(B)
    ]
    res = run_bass_kernel_spmd(_get_nc(), in_maps, list(range(B)), trace=trace)
    out = np.stack([res.results[b]["OUT"] for b in range(B)])
    return out.reshape(B, C, H, W).astype(np.float32), res


def kernel(**inputs) -> np.ndarray:
    return run(inputs)[0]



# revision 4
# speedup vs baseline: 1.3462x; 1.3462x over previous
"""Trainium2 Bass kernel for the DFS-Mixer style-attention module.

Computation (per batch b):
    dot[k,c]  = sum_hw CT[c,hw] * CR[k,c,hw]
    norm[k,c] = sqrt(sum_hw CR[k,c,hw]^2)
    w[.,c]    = softmax_k(2 * dot[.,c] / norm[.,c])
    out[c,hw] = sum_k IR[k,c,hw] * w[k,c]

Sharding: data-parallel over batch B=8 across the 8 NeuronCores (one b per
core, no cross-core communication).

The module is DMA-bound (reads all of IR and CR once, tiny compute per
byte), so inputs are cast to bf16 on the host: per-core traffic drops from
75.5 MB (fp32) to 37.8 MB, which halves the DMA-roofline floor.  The
rel-err budget (2e-2) dwarfs bf16's ~0.4 % element error; dot/norm/softmax
accumulate in fp32 throughout.

Per-core layout: C=256 tiled as 2 x 128 SBUF partitions, HW=4096 on the
free axis ([128, 4096] bf16 tiles, 8 KB per-partition DMA rows).

- The two c-tiles' work is INTERLEAVED on the in-order sync queue:
  [CT, CR(t0,*), then alternating CR(t1,k) / IR(t0,k), then IR(t1,*)].
  Phase 1 consumers (ACT+DVE) and phase 3's TensorE then drain two
  independent streams concurrently, so a hiccup in either consumer never
  lets the DMA queues run dry.
- Phase 1, per CR tile (2048-wide halves; both engines run measurably
  faster on 2048-wide ops than 4096-wide ones):
  DVE (2x-1p double-pump on bf16): fused dot via scalar_tensor_tensor on
  both halves + self-mult square on half 1 (~1.6 us/tile);
  ACT (no 16-bit speedup, 1 elem/cyc): Square-with-accumulate on half 0
  (~1.7 us/tile).  Both stay under the ~2.7 us/tile best-case bf16 DMA
  cadence, so the slot-release -> next-issue loop never convoys.
- Phase 3: TensorE folds all k with bf16 matmuls (1 cyc/row) against the
  128x128 diagonal diag(w[:,k]), accumulating across k in PSUM (8 banks =
  the full [128, 4096] fp32 accumulator).  ACT drains finished PSUM
  quarters to bf16 staging (free downcast); outputs ride the scalar queue
  so the sync queue never head-of-line blocks on epilogue dependencies.
- Output is written bf16 (halves epilogue DMA) and upcast on the host.

CR rides the sync queue; IR rides the GpSimd SWDGE queue; CT/outputs ride
the scalar queue.  Three independent in-order issue queues mean a full
slot pool in one stream never head-of-line blocks the others.
"""

import os
import sys

import numpy as np


def _import_concourse():
    try:
        import concourse.bass  # noqa: F401
    except ImportError:
        for p in ("/opt/trn_rl_repo", "/root/.axon_site/_ro/trn_rl_repo"):
            if os.path.isdir(p) and p not in sys.path:
                sys.path.insert(0, p)
        import concourse.bass  # noqa: F401


_import_concourse()

import ml_dtypes  # noqa: E402

import concourse.bass as bass  # noqa: E402
import concourse.mybir as mybir  # noqa: E402
from concourse import tile  # noqa: E402
from concourse.bass_utils import run_bass_kernel_spmd  # noqa: E402
from concourse.vector_clock import ScopedClock, VectorClock  # noqa: E402


def _split_multiwait_bir(bir: bytes) -> bytes:
    """The neuronxcc walrus in this container encodes at most ONE sync-wait
    per instruction; Tile emits several.  Hoist extra waits onto same-engine
    NoOp instructions inserted immediately before the original instruction
    (engines execute in order, so waiting earlier on the same engine is
    semantically identical).  Sem *updates* are left untouched (a DMA's
    completion-inc cannot move to a sequencer NoOp)."""
    import json

    j = json.loads(bir)
    ctr = 0
    for f in j.get("functions", []):
        for bb in f.get("blocks", []):
            out_insts = []
            for ins in bb.get("instructions", []):
                si = ins.get("sync_info")
                waits = (si or {}).get("on_wait") or []
                if len(waits) > 1:
                    for w in waits[:-1]:
                        ctr += 1
                        nop = {
                            "engine": ins["engine"],
                            "ins": [],
                            "outs": [],
                            "name": f"waitsplit-{ctr}",
                            "opcode": "NoOp",
                            "sync_info": {"on_update": [], "on_wait": [w]},
                        }
                        if "debug" in ins:
                            nop["debug"] = ins["debug"]
                        out_insts.append(nop)
                    si["on_wait"] = [waits[-1]]
                out_insts.append(ins)
            bb["instructions"] = out_insts
    return json.dumps(j).encode()


_orig_to_json_bytes = bass.Bass.to_json_bytes


def _patched_to_json_bytes(self, *a, **kw):
    return _split_multiwait_bir(_orig_to_json_bytes(self, *a, **kw))


bass.Bass.to_json_bytes = _patched_to_json_bytes


def _patched_drain_and_barrier(self, tick_clock, wait_clock):
    # Stock TileContext exit emits one Drain waiting on every used semaphore,
    # which this walrus rejects ("Too many sync wait commands").  Emit one
    # Drain per semaphore instead.
    gc = tick_clock.global_clock
    n = len(gc)
    nonzero = [p for p in range(n) if gc[p] > 0] or [0]
    for p in nonzero:
        d = self.nc.sync.drain()
        vec = [gc[q] if q == p else 0 for q in range(n)]
        wait_clock.add_sem_waits(d.ins, ScopedClock({None: VectorClock(vec)}))
    self.nc.all_engine_barrier()
    popped = self.nc._tile_sem_poison_stack.pop()
    assert popped is self._sem_poison
    self.nc.clear_and_free_semaphores(list(self.sems.allocated().values()))
    self.nc.all_engine_barrier()


tile.TileContext._drain_and_barrier = _patched_drain_and_barrier

FP = mybir.dt.float32
BF = mybir.dt.bfloat16
B, K, C, H, W = 8, 8, 256, 64, 64
HW = H * W
P = 128                 # SBUF partitions
NCT = C // P            # 2 c-tiles per core
MMN = 512               # moving free dim per matmul (= one PSUM bank of f32)
QN = 1024               # tail quarter width (last k of last c-tile)
HH = 2048               # consumer half-tile width

_AF = mybir.ActivationFunctionType
_OP = mybir.AluOpType
_X = mybir.AxisListType.X


def build_nc() -> bass.Bass:
    nc = bass.Bass()
    IR = nc.declare_dram_parameter("IR", [K, C, HW], BF, isOutput=False)
    CR = nc.declare_dram_parameter("CR", [K, C, HW], BF, isOutput=False)
    CT = nc.declare_dram_parameter("CT", [C, HW], BF, isOutput=False)
    OUT = nc.declare_dram_parameter("OUT", [C, HW], BF, isOutput=True)

    with tile.TileContext(nc) as tc:
        with (
            tc.tile_pool(name="ctp", bufs=1) as ct_pool,
            tc.tile_pool(name="crp", bufs=6) as cr_pool,
            tc.tile_pool(name="irp", bufs=6) as ir_pool,
            tc.tile_pool(name="ir7", bufs=2) as ir7_pool,
            tc.tile_pool(name="obp", bufs=3) as ob_pool,
            tc.tile_pool(name="snk", bufs=1) as sink_pool,
            tc.tile_pool(name="sml", bufs=1) as small,
            tc.tile_pool(name="psp", bufs=1, space="PSUM") as psum_pool,
        ):
            acc = psum_pool.tile([P, HW], FP, name="acc")
            # Dead destinations for the elementwise outputs of the phase-1
            # reduction ops (only the accum_out side-outputs are live).  One
            # sink per engine so ACT and DVE never serialize on a WAW.
            sinka = sink_pool.tile([P, HH], BF, name="sinka")
            sinkd = sink_pool.tile([P, HH], BF, name="sinkd")

            # Diagonal ones mask, built once: mask[p, f] = (p == f).
            ones_t = small.tile([P, P], FP, name="ones_t")
            nc.vector.memset(ones_t[:], 1.0)
            mask = small.tile([P, P], FP, name="mask")
            nc.gpsimd.affine_select(
                mask[:],
                ones_t[:],
                pattern=[[-1, P]],
                compare_op=_OP.is_equal,
                fill=0.0,
                base=0,
                channel_multiplier=1,
            )

            # Content-target features stay resident in SBUF (reused by all
            # k).  Issued from the scalar queue so they overlap the sync
            # queue's first CR issues.
            ct_tiles = []
            for t in range(NCT):
                ctt = ct_pool.tile([P, HW], BF, name=f"ct{t}", tag=f"ct{t}")
                nc.scalar.dma_start(out=ctt[:], in_=CT[t * P:(t + 1) * P, :])
                ct_tiles.append(ctt)

            dot2s, sqas, sqds, ws, wms = [], [], [], [None] * NCT, [None] * NCT

            def phase1_k(t, k):
                """Load CR[k] for c-tile t; DVE (bf16 2x mode) does the
                fused dot on both halves + the square on half 1; ACT does
                the square on half 0.  fp32 per-partition accumulators.
                All elementwise outputs go to dead sinks (crt is never
                written after the DMA), so ACT and DVE run with no
                cross-engine ordering between them."""
                cs = slice(t * P, (t + 1) * P)
                crt = cr_pool.tile([P, HW], BF, name="crt", tag="cr")
                nc.sync.dma_start(out=crt[:], in_=CR[k, cs, :])
                h0 = slice(0, HH)
                h1 = slice(HH, 2 * HH)
                nc.scalar.activation(
                    out=sinka[:], in_=crt[:, h0], func=_AF.Square,
                    accum_out=sqas[t][:, k:k + 1],
                )
                nc.vector.scalar_tensor_tensor(
                    out=sinkd[:], in0=crt[:, h1], scalar=1.0,
                    in1=crt[:, h1],
                    op0=_OP.mult, op1=_OP.mult,
                    accum_out=sqds[t][:, k:k + 1],
                )
                for h, hs in ((0, h0), (1, h1)):
                    nc.vector.scalar_tensor_tensor(
                        out=sinkd[:], in0=crt[:, hs], scalar=1.0,
                        in1=ct_tiles[t][:, hs],
                        op0=_OP.mult, op1=_OP.mult,
                        accum_out=dot2s[t][:, k * 2 + h:k * 2 + h + 1],
                    )

            def softmax(t):
                """Combine half-accumulators, softmax over K, build diag
                weight matrices (tiny [128, K]/[128, 128] ops)."""
                dot2 = dot2s[t]
                dot = small.tile([P, K], FP, name=f"dotc{t}", tag=f"dotc{t}")
                sq = small.tile([P, K], FP, name=f"sqc{t}", tag=f"sqc{t}")
                nc.vector.tensor_add(
                    dot[:], dot2[:, 0:2 * K:2], dot2[:, 1:2 * K:2]
                )
                nc.vector.tensor_add(sq[:], sqas[t][:], sqds[t][:])
                norm = small.tile([P, K], FP, name=f"norm{t}", tag=f"norm{t}")
                nc.scalar.activation(norm[:], sq[:], func=_AF.Sqrt)
                rnorm = small.tile([P, K], FP, name=f"rnorm{t}", tag=f"rn{t}")
                nc.vector.reciprocal(rnorm[:], norm[:])
                sim = small.tile([P, K], FP, name=f"sim{t}", tag=f"sim{t}")
                nc.vector.tensor_mul(sim[:], dot[:], rnorm[:])
                mx = small.tile([P, 1], FP, name=f"mx{t}", tag=f"mx{t}")
                nc.vector.reduce_max(mx[:], sim[:], axis=_X)
                nbias = small.tile([P, 1], FP, name=f"nb{t}", tag=f"nb{t}")
                nc.vector.tensor_scalar_mul(nbias[:], mx[:], -2.0)
                e = small.tile([P, K], FP, name=f"e{t}", tag=f"e{t}")
                nc.scalar.activation(
                    e[:], sim[:], func=_AF.Exp, bias=nbias[:, 0:1], scale=2.0
                )
                s = small.tile([P, 1], FP, name=f"s{t}", tag=f"s{t}")
                nc.vector.reduce_sum(s[:], e[:], axis=_X)
                rs = small.tile([P, 1], FP, name=f"rs{t}", tag=f"rs{t}")
                nc.vector.reciprocal(rs[:], s[:])
                w = small.tile([P, K], FP, name=f"w{t}", tag=f"w{t}")
                nc.vector.tensor_scalar_mul(w[:], e[:], rs[:, 0:1])
                ws[t] = w
                wmt = []
                for k in range(K):
                    wm = small.tile(
                        [P, P], BF, name=f"wm{t}{k}", tag=f"wm{t}{k}"
                    )
                    nc.vector.tensor_scalar_mul(wm[:], mask[:], w[:, k:k + 1])
                    wmt.append(wm)
                wms[t] = wmt

            def drain(t, q, on_dve=False):
                """Copy a finished PSUM quarter to bf16 staging (ACT, or DVE
                for alternating tail quarters so the final drains run on two
                engines in parallel); the output DMA streams it out from
                the scalar queue."""
                cs = slice(t * P, (t + 1) * P)
                qs = slice(q * QN, (q + 1) * QN)
                ob = ob_pool.tile([P, QN], BF, name="ob", tag="ob")
                if on_dve:
                    nc.vector.tensor_scalar_add(ob[:], acc[:, qs], 0.0)
                else:
                    nc.scalar.activation(ob[:], acc[:, qs], func=_AF.Copy)
                nc.scalar.dma_start(out=OUT[cs, qs], in_=ob[:])

            def phase3_k(t, k):
                """Load IR[k] for c-tile t and fold it into the PSUM
                accumulator with bf16 diagonal matmuls.  The last k closes
                each bank group and drains it."""
                cs = slice(t * P, (t + 1) * P)
                last = k == K - 1
                if not (last and t == NCT - 1):
                    irt = ir_pool.tile([P, HW], BF, name="irt", tag="ir")
                    nc.gpsimd.dma_start(out=irt[:], in_=IR[k, cs, :])
                    for j in range(HW // MMN):
                        col = j * MMN
                        nc.tensor.matmul(
                            acc[:, col:col + MMN],
                            wms[t][k][:],
                            irt[:, col:col + MMN],
                            start=(k == 0),
                            stop=last,
                        )
                    if last:
                        for q in range(HW // QN):
                            drain(t, q)
                else:
                    # Very last input: quarter loads so the tail after the
                    # final byte is two short matmuls + copy + small DMA,
                    # with drains alternating across engines.
                    ir7s = []
                    for q in range(HW // QN):
                        qs = slice(q * QN, (q + 1) * QN)
                        ir7 = ir7_pool.tile([P, QN], BF, name="ir7", tag="ir7")
                        nc.gpsimd.dma_start(out=ir7[:], in_=IR[K - 1, cs, qs])
                        ir7s.append(ir7)
                    for q in range(HW // QN):
                        for jj in range(QN // MMN):
                            col = q * QN + jj * MMN
                            nc.tensor.matmul(
                                acc[:, col:col + MMN],
                                wms[t][K - 1][:],
                                ir7s[q][:, jj * MMN:(jj + 1) * MMN],
                                start=False,
                                stop=True,
                            )
                        drain(t, q)

            for t in range(NCT):
                dot2s.append(
                    small.tile([P, 2 * K], FP, name=f"dot{t}", tag=f"dot{t}")
                )
                sqas.append(
                    small.tile([P, K], FP, name=f"sqa{t}", tag=f"sqa{t}")
                )
                sqds.append(
                    small.tile([P, K], FP, name=f"sqd{t}", tag=f"sqd{t}")
                )

            # ---- Interleaved schedule ----
            # CR(t1) / IR(t0) alternate on the sync queue: phase 1's ACT+DVE
            # and phase 3's TensorE drain two independent streams, so a
            # transient in either consumer never idles the DMA engines.
            for k in range(K):
                phase1_k(0, k)
            phase1_k(1, 0)
            softmax(0)
            for k in range(1, K):
                phase3_k(0, k - 1)
                phase1_k(1, k)
            phase3_k(0, K - 1)
            softmax(1)
            for k in range(K):
                phase3_k(1, k)

    return nc


_NC_CACHE = None


def _get_nc() -> bass.Bass:
    global _NC_CACHE
    if _NC_CACHE is None:
        _NC_CACHE = build_nc()
    return _NC_CACHE


def run(inputs: dict, trace: bool = False):
    """Shard over B, run on 8 cores, gather. Returns (output, BassKernelResults)."""
    bf16 = ml_dtypes.bfloat16
    ir = np.asarray(inputs["IR_features"], dtype=np.float32).astype(bf16)
    cr = np.asarray(inputs["CR_features"], dtype=np.float32).astype(bf16)
    ct = np.asarray(inputs["CT_feature"], dtype=np.float32).astype(bf16)
    assert ir.shape == (B, K, C, H, W) and cr.shape == (B, K, C, H, W)
    assert ct.shape == (B, C, H, W)

    in_maps = [
        {
            "IR": np.ascontiguousarray(ir[b].reshape(K, C, HW)),
            "CR": np.ascontiguousarray(cr[b].reshape(K, C, HW)),
            "CT": np.ascontiguousarray(ct[b].reshape(C, HW)),
        }
        for b in range(B)
    ]
    res = run_bass_kernel_spmd(_get_nc(), in_maps, list(range(B)), trace=trace)
    out = np.stack([res.results[b]["OUT"] for b in range(B)])
    return out.reshape(B, C, H, W).astype(np.float32), res


def kernel(**inputs) -> np.ndarray:
    return run(inputs)[0]
